# revision 4
# baseline (speedup 1.0000x reference)
"""CornerNet module (2x NonLocal attention + 6 conv heads) on 8 trn2 NeuronCores.

Distribution: core c -> batch b=c//4, block t=(c//2)%2 (tl/br), half h=c%2.
Each core pair (2b+t) computes one NonLocal block's attention split by
m-groups (the softmax axis=1 normalizes over n for fixed m; with the
permuted enumeration n = 72*q + r the CornerNet raw-reshape scramble is
layout-free).  Partial y^T is AllReduced within the pair, then each core
computes the full W-conv + residual (z) and half the head output channels
of all 3 heads (conv3x3 o-half; conv1x1 partial sums added on host).

Self-contained: hardcodes shapes/sharding; builds+compiles the Bass program
once per process and runs it via run_bass_kernel_spmd on cores 0-7.
"""
import sys

if "/opt/trn_rl_repo" not in sys.path:
    sys.path.insert(0, "/opt/trn_rl_repo")

import numpy as np
from contextlib import ExitStack

import concourse.bass as bass
import concourse.tile as tile
from concourse import bacc, mybir
from concourse.bass_utils import run_bass_kernel_spmd

DT = mybir.dt
F32 = DT.float32
BF16 = DT.bfloat16
AF = mybir.ActivationFunctionType
AX = mybir.AxisListType
ALU = mybir.AluOpType

P = 128          # partitions / bottleneck channels
KT = 2           # 256 input channels = 2 k-tiles
NPIX = 9216      # 96*96
R = 72           # n-groups (9216 = 72*128)
RH = 36          # m-groups owned per core (half of 72)
NPIXH = RH * P   # 4608
NCH = 18         # 512-wide chunks over NPIX
PADLEN = 9802    # 99 + 98*98 + 99 padded z row layout
NPAD = 9604      # 98*98 padded grid positions
HEADS = (("heat", 80), ("tag", 1), ("regr", 2))
N_CORES = 8
GROUPS = [[0, 1], [2, 3], [4, 5], [6, 7]]

_NC = None


def _build():
    nc = bacc.Bacc("TRN2", target_bir_lowering=False, debug=False,
                   num_devices=N_CORES)

    # ---- external inputs (per-core data encodes the (b, t, h) assignment) ----
    x_d = nc.dram_tensor("x", [P, KT, NPIX], F32, kind="ExternalInput")
    xm_d = nc.dram_tensor("xm", [P, KT, NPIXH], F32, kind="ExternalInput")
    xg_d = nc.dram_tensor("xg", [P, KT, NPIXH], F32, kind="ExternalInput")
    thwT_d = nc.dram_tensor("thwT", [P, KT, P], F32, kind="ExternalInput")
    phwT_d = nc.dram_tensor("phwT", [P, KT, P], F32, kind="ExternalInput")
    gwT_d = nc.dram_tensor("gwT", [P, KT, P], F32, kind="ExternalInput")
    wwT_d = nc.dram_tensor("wwT", [P, KT, P], F32, kind="ExternalInput")
    thb_d = nc.dram_tensor("thb", [1, P], F32, kind="ExternalInput")
    phb_d = nc.dram_tensor("phb", [P, 1], F32, kind="ExternalInput")
    gb_d = nc.dram_tensor("gb", [P, 1], F32, kind="ExternalInput")
    wb_d = nc.dram_tensor("wb", [P, KT], F32, kind="ExternalInput")
    c3w_d, c3b_d, c1w_d, c1b_d, out_d = {}, {}, {}, {}, {}
    for hname, dd in HEADS:
        c3w_d[hname] = nc.dram_tensor(f"c3w_{hname}", [P, KT, 9, P], F32,
                                      kind="ExternalInput")
        c3b_d[hname] = nc.dram_tensor(f"c3b_{hname}", [P, 1], F32,
                                      kind="ExternalInput")
        c1w_d[hname] = nc.dram_tensor(f"c1w_{hname}", [P, dd], F32,
                                      kind="ExternalInput")
        c1b_d[hname] = nc.dram_tensor(f"c1b_{hname}", [dd, 1], F32,
                                      kind="ExternalInput")
        out_d[hname] = nc.dram_tensor(f"out_{hname}", [dd, NPIX], F32,
                                      kind="ExternalOutput")

    # internal DRAM bounce for the pairwise AllReduce of partial y^T
    cc_in = nc.dram_tensor("cc_in", [P, NPIX], BF16)
    cc_out = nc.dram_tensor("cc_out", [P, NPIX], BF16)

    with tile.TileContext(nc) as tc, ExitStack() as octx:
        const = octx.enter_context(tc.tile_pool(name="const", bufs=1))

        def load_cast(dram, shape, nm):
            f = const.tile(shape, F32, tag=f"f32_{nm}")
            nc.sync.dma_start(f, dram.ap())
            b16 = const.tile(shape, BF16, tag=f"b16_{nm}")
            nc.vector.tensor_copy(b16, f)
            return b16

        thwT = load_cast(thwT_d, [P, KT, P], "thwT")
        phwT = load_cast(phwT_d, [P, KT, P], "phwT")
        gwT = load_cast(gwT_d, [P, KT, P], "gwT")
        wwT = load_cast(wwT_d, [P, KT, P], "wwT")
        thb = load_cast(thb_d, [1, P], "thb")
        ones1 = const.tile([1, P], BF16)
        nc.vector.memset(ones1, 1.0)
        phb = const.tile([P, 1], F32)
        nc.sync.dma_start(phb, phb_d.ap())
        gb = const.tile([P, 1], F32)
        nc.sync.dma_start(gb, gb_d.ap())
        wb = const.tile([P, KT], F32)
        nc.sync.dma_start(wb, wb_d.ap())

        attn_stack = ExitStack()
        feat = attn_stack.enter_context(tc.tile_pool(name="feat", bufs=1))
        Tp = feat.tile([P, R, P], BF16)      # theta^T groups (option-B conv out)
        Pm = feat.tile([P, RH, P], BF16)     # phi columns for own m-groups
        G = feat.tile([P, RH, P], BF16)      # g tiles for own m-groups
        yT = attn_stack.enter_context(tc.tile_pool(name="yT", bufs=1)).tile(
            [P, NPIX], F32)

        # =========================== prelim convs ===========================
        with ExitStack() as ctx:
            xpool = ctx.enter_context(tc.tile_pool(name="xbf", bufs=1))
            xbf = xpool.tile([P, KT, NPIX], BF16)
            xmbf = xpool.tile([P, KT, NPIXH], BF16)
            xgbf = xpool.tile([P, KT, NPIXH], BF16)
            stage = ctx.enter_context(tc.tile_pool(name="stage", bufs=4))
            psA = ctx.enter_context(tc.tile_pool(name="psA", bufs=4,
                                                 space="PSUM"))
            psB = ctx.enter_context(tc.tile_pool(name="psB", bufs=2,
                                                 space="PSUM"))
            for ci in range(NCH):
                s = stage.tile([P, KT, 512], F32, tag="xs")
                nc.sync.dma_start(s, x_d[:, :, 512 * ci:512 * (ci + 1)])
                nc.vector.tensor_copy(xbf[:, :, 512 * ci:512 * (ci + 1)], s)
            for ci in range(9):
                s = stage.tile([P, KT, 512], F32, tag="xs")
                nc.sync.dma_start(s, xm_d[:, :, 512 * ci:512 * (ci + 1)])
                nc.vector.tensor_copy(xmbf[:, :, 512 * ci:512 * (ci + 1)], s)
                s2 = stage.tile([P, KT, 512], F32, tag="xs")
                nc.sync.dma_start(s2, xg_d[:, :, 512 * ci:512 * (ci + 1)])
                nc.vector.tensor_copy(xgbf[:, :, 512 * ci:512 * (ci + 1)], s2)

            # T' = x^T @ theta_w^T (+ theta_b via rank-1 ones trick)
            for r in range(R):
                ps = psA.tile([P, P], F32, tag="tps")
                for k in range(KT):
                    nc.tensor.matmul(ps, xbf[:, k, P * r:P * (r + 1)],
                                     thwT[:, k, :], start=(k == 0), stop=False)
                nc.tensor.matmul(ps, ones1, thb, start=False, stop=True)
                nc.vector.tensor_copy(Tp[:, r, :], ps)
            # P (phi) over own m-columns; G (g) over own pixel half
            for ci in range(9):
                ps = psB.tile([P, 512], F32, tag="pps")
                for k in range(KT):
                    nc.tensor.matmul(ps, phwT[:, k, :],
                                     xmbf[:, k, 512 * ci:512 * (ci + 1)],
                                     start=(k == 0), stop=(k == KT - 1))
                nc.scalar.activation(Pm[:, 4 * ci:4 * (ci + 1), :], ps,
                                     AF.Identity, bias=phb)
                ps2 = psB.tile([P, 512], F32, tag="pps")
                for k in range(KT):
                    nc.tensor.matmul(ps2, gwT[:, k, :],
                                     xgbf[:, k, 512 * ci:512 * (ci + 1)],
                                     start=(k == 0), stop=(k == KT - 1))
                nc.scalar.activation(G[:, 4 * ci:4 * (ci + 1), :], ps2,
                                     AF.Identity, bias=gb)

        # ====================== attention main pipeline ======================
        with ExitStack() as ctx:
            epool = ctx.enter_context(tc.tile_pool(name="E", bufs=5))
            zpool = ctx.enter_context(tc.tile_pool(name="zstat", bufs=8))
            gpool = ctx.enter_context(tc.tile_pool(name="gtile", bufs=8))
            ps1 = ctx.enter_context(tc.tile_pool(name="ps1", bufs=2,
                                                 space="PSUM"))
            ps2 = ctx.enter_context(tc.tile_pool(name="ps2", bufs=2,
                                                 space="PSUM"))
            for q in range(9):
                epanels, gtiles = [], []
                for j in range(4):
                    rl = 4 * q + j
                    E = epool.tile([P, NPIX], BF16, tag="E")
                    Zp = zpool.tile([P, 6], F32, tag="zp")
                    for g6 in range(6):
                        ps = ps1.tile([P, 1536], F32, tag="s")
                        for c3 in range(3):
                            ci = 3 * g6 + c3
                            nc.tensor.matmul(
                                ps[:, 512 * c3:512 * (c3 + 1)],
                                Pm[:, rl, :],
                                Tp[:, 4 * ci:4 * (ci + 1), :],
                                start=True, stop=True)
                        nc.scalar.activation(
                            E[:, 1536 * g6:1536 * (g6 + 1)], ps, AF.Exp,
                            accum_out=Zp[:, g6:g6 + 1])
                    Z = zpool.tile([P, 1], F32, tag="z")
                    nc.vector.reduce_sum(Z, Zp, axis=AX.X)
                    Zi = zpool.tile([P, 1], F32, tag="zi")
                    nc.vector.reciprocal(Zi, Z)
                    gt = gpool.tile([P, P], BF16, tag="g")
                    nc.vector.tensor_scalar_mul(gt, G[:, rl, :], Zi)
                    epanels.append(E)
                    gtiles.append(gt)
                for ci in range(NCH):
                    ps = ps2.tile([P, 512], F32, tag="s2")
                    for j in range(4):
                        nc.tensor.matmul(
                            ps, gtiles[j],
                            epanels[j][:, 512 * ci:512 * (ci + 1)],
                            start=(j == 0), stop=(j == 3))
                    if q == 0:
                        nc.vector.tensor_copy(
                            yT[:, 512 * ci:512 * (ci + 1)], ps)
                    else:
                        nc.vector.tensor_tensor(
                            yT[:, 512 * ci:512 * (ci + 1)],
                            yT[:, 512 * ci:512 * (ci + 1)], ps, ALU.add)

        # ============== pair AllReduce of y^T =============
        with ExitStack() as ctx:
            yp = ctx.enter_context(tc.tile_pool(name="y16pool", bufs=1))
            y16 = yp.tile([P, NPIX], BF16, tag="y16")
            nc.vector.tensor_copy(y16, yT)
            nc.sync.dma_start(cc_in[:, :], y16)
            with tc.tile_critical():
                cc_sem = nc.alloc_semaphore("cc_sem")
                nc.gpsimd.collective_compute(
                    "AllReduce", ALU.add, replica_groups=GROUPS,
                    ins=[cc_in.ap()], outs=[cc_out.ap()],
                ).then_inc(cc_sem, 1)
                nc.gpsimd.wait_ge(cc_sem, 1)
        attn_stack.close()

        # ====== un-scramble to Y + W conv + residual -> padded z ======
        zf = octx.enter_context(tc.tile_pool(name="zfeat", bufs=1))
        z_sb = zf.tile([P, KT, PADLEN], BF16)
        nc.vector.memset(z_sb, 0.0)
        with ExitStack() as ctx:
            Ypool = ctx.enter_context(tc.tile_pool(name="Y", bufs=1))
            Y = Ypool.tile([P, NPIX], BF16)
            sp = ctx.enter_context(tc.tile_pool(name="spool", bufs=8))
            for r2 in range(R):
                stg = sp.tile([P, P], BF16, tag="tstg")
                nc.sync.dma_start_transpose(stg,
                                            cc_out[:, P * r2:P * (r2 + 1)])
                nc.vector.tensor_copy(Y[:, P * r2:P * (r2 + 1)], stg)
            xr = ctx.enter_context(tc.tile_pool(name="xres", bufs=6))
            tpool = ctx.enter_context(tc.tile_pool(name="wtmp", bufs=4))
            wps = ctx.enter_context(tc.tile_pool(name="wps", bufs=4,
                                                 space="PSUM"))
            for ot in range(KT):
                for rc in range(24):
                    a0 = 384 * rc
                    ps = wps.tile([P, 384], F32, tag="w")
                    nc.tensor.matmul(ps, wwT[:, ot, :], Y[:, a0:a0 + 384],
                                     start=True, stop=True)
                    xs = xr.tile([P, 384], F32, tag="x")
                    nc.sync.dma_start(xs, x_d[:, ot, a0:a0 + 384])
                    tmp = tpool.tile([P, 384], F32, tag="t")
                    nc.scalar.activation(tmp, ps, AF.Identity,
                                         bias=wb[:, ot:ot + 1])
                    dst = z_sb[:, ot, 198 + 392 * rc:198 + 392 * (rc + 1)]
                    dst = dst.rearrange("p (r c) -> p r c", c=98)[:, :, :96]
                    nc.vector.tensor_tensor(
                        dst, tmp.rearrange("p (r c) -> p r c", c=96),
                        xs.rearrange("p (r c) -> p r c", c=96), ALU.add)

        # ============================== heads ===============================
        with ExitStack() as ctx:
            wstage = ctx.enter_context(tc.tile_pool(name="wstage", bufs=2))
            wpool = ctx.enter_context(tc.tile_pool(name="hw", bufs=2))
            hpool = ctx.enter_context(tc.tile_pool(name="hbuf", bufs=2))
            opool = ctx.enter_context(tc.tile_pool(name="obuf", bufs=1))
            hps = ctx.enter_context(tc.tile_pool(name="hps", bufs=2,
                                                 space="PSUM"))
            ops = ctx.enter_context(tc.tile_pool(name="ops", bufs=2,
                                                 space="PSUM"))
            for hname, dd in HEADS:
                wf = wstage.tile([P, KT, 9, P], F32, tag="wf")
                nc.sync.dma_start(wf, c3w_d[hname].ap())
                c3w = wpool.tile([P, KT, 9, P], BF16, tag="c3w")
                nc.vector.tensor_copy(c3w, wf)
                c3b = wpool.tile([P, 1], F32, tag="c3b")
                nc.sync.dma_start(c3b, c3b_d[hname].ap())
                c1f = wstage.tile([P, dd], F32, tag=f"c1f_{hname}")
                nc.sync.dma_start(c1f, c1w_d[hname].ap())
                c1w = wpool.tile([P, dd], BF16, tag=f"c1w_{hname}")
                nc.vector.tensor_copy(c1w, c1f)
                c1b = wpool.tile([dd, 1], F32, tag=f"c1b_{hname}")
                nc.sync.dma_start(c1b, c1b_d[hname].ap())

                h_sb = hpool.tile([P, NPAD], BF16, tag="h")
                for ci in range(19):
                    w = 512 if ci < 18 else NPAD - 512 * 18
                    ps = hps.tile([P, 512], F32, tag="h")
                    first = True
                    for k in range(KT):
                        for tap in range(9):
                            off = (99 + 512 * ci + 98 * (tap // 3 - 1)
                                   + (tap % 3 - 1))
                            nc.tensor.matmul(
                                ps[:, :w], c3w[:, k, tap, :],
                                z_sb[:, k, off:off + w],
                                start=first, stop=(k == KT - 1 and tap == 8))
                            first = False
                    nc.scalar.activation(h_sb[:, 512 * ci:512 * ci + w],
                                         ps[:, :w], AF.Relu, bias=c3b)
                out_sb = opool.tile([P, NPAD], F32, tag="o")
                for ci in range(19):
                    w = 512 if ci < 18 else NPAD - 512 * 18
                    ps = ops.tile([P, 512], F32, tag="o")
                    nc.tensor.matmul(ps[:dd, :w], c1w,
                                     h_sb[:, 512 * ci:512 * ci + w],
                                     start=True, stop=True)
                    nc.scalar.activation(out_sb[:dd, 512 * ci:512 * ci + w],
                                         ps[:dd, :w], AF.Identity, bias=c1b)
                src = out_sb[:dd, 99:99 + 9408]
                src = src.rearrange("d (r c) -> d r c", c=98)[:, :, :96]
                nc.sync.dma_start(
                    out_d[hname].ap().rearrange("d (r c) -> d r c", c=96),
                    src)

    nc.compile()
    return nc


def _get_nc():
    global _NC
    if _NC is None:
        _NC = _build()
    return _NC


def _make_inmaps(features, params):
    features = np.asarray(features, dtype=np.float32)
    ins = []
    # m-column gather index for P: col 128*rl + q -> pixel 72*q + 36*h + rl
    for c in range(N_CORES):
        b = c // 4
        t = "tl" if ((c // 2) % 2) == 0 else "br"
        h = c % 2
        x = np.ascontiguousarray(
            features[b].reshape(KT, P, NPIX).transpose(1, 0, 2))
        idx = (72 * np.arange(P)[None, :] + 36 * h
               + np.arange(RH)[:, None]).reshape(-1)
        xm = np.ascontiguousarray(
            features[b].reshape(256, NPIX)[:, idx]
            .reshape(KT, P, NPIXH).transpose(1, 0, 2))
        xg = np.ascontiguousarray(
            features[b].reshape(256, NPIX)[:, NPIXH * h:NPIXH * (h + 1)]
            .reshape(KT, P, NPIXH).transpose(1, 0, 2))
        nl = params[f"{t}_nl"]
        d = {
            "x": x, "xm": xm, "xg": xg,
            "thwT": np.ascontiguousarray(
                np.asarray(nl["theta_w"], np.float32).T
                .reshape(KT, P, P).transpose(1, 0, 2)),
            "phwT": np.ascontiguousarray(
                np.asarray(nl["phi_w"], np.float32).T
                .reshape(KT, P, P).transpose(1, 0, 2)),
            "gwT": np.ascontiguousarray(
                np.asarray(nl["g_w"], np.float32).T
                .reshape(KT, P, P).transpose(1, 0, 2)),
            "wwT": np.ascontiguousarray(
                np.asarray(nl["W_w"], np.float32)
                .reshape(KT, P, P).transpose(2, 0, 1)),
            "thb": np.asarray(nl["theta_b"], np.float32).reshape(1, P),
            "phb": np.asarray(nl["phi_b"], np.float32).reshape(P, 1),
            "gb": np.asarray(nl["g_b"], np.float32).reshape(P, 1),
            "wb": np.ascontiguousarray(
                np.asarray(nl["W_b"], np.float32).reshape(KT, P).T),
        }
        for hname, dd, pkey in (("heat", 80, f"{t}_heat"),
                                ("tag", 1, f"{t}_tags"),
                                ("regr", 2, f"{t}_regr")):
            hp = params[pkey]
            c3 = np.asarray(hp["c3_w"], np.float32)  # [256, 256, 3, 3]
            c3 = c3[P * h:P * (h + 1)].reshape(P, KT, P, 9)
            d[f"c3w_{hname}"] = np.ascontiguousarray(
                c3.transpose(2, 1, 3, 0))          # [i_l, k, tap, o_l]
            d[f"c3b_{hname}"] = np.asarray(
                hp["c3_b"], np.float32)[P * h:P * (h + 1)].reshape(P, 1)
            d[f"c1w_{hname}"] = np.ascontiguousarray(
                np.asarray(hp["c1_w"], np.float32)[:, P * h:P * (h + 1)].T)
            cb = np.asarray(hp["c1_b"], np.float32).reshape(dd, 1)
            d[f"c1b_{hname}"] = cb if h == 0 else np.zeros_like(cb)
        ins.append(d)
    return ins


def _assemble(results):
    outs = []
    for oname, dd in (("heat", 80), ("tag", 1), ("regr", 2)):
        for t in range(2):  # tl, br
            full = []
            for b in range(2):
                c0 = 4 * b + 2 * t
                part = (results[c0][f"out_{oname}"]
                        + results[c0 + 1][f"out_{oname}"])
                full.append(part.reshape(dd, 96, 96))
            outs.append(np.stack(full).astype(np.float32))
    # order: tl_heat, br_heat, tl_tag, br_tag, tl_regr, br_regr
    return tuple(outs)


def run(features, params):
    nc = _get_nc()
    ins = _make_inmaps(features, params)
    res = run_bass_kernel_spmd(nc, ins, core_ids=list(range(N_CORES)))
    return _assemble(res.results), res


def kernel(features, params):
    out, _ = run(features, params)
    return out
